# revision 17
# baseline (speedup 1.0000x reference)
"""CASCADES adapter (moe_routing) on 8 TRN2 NeuronCores.

Reference computation (B=4, S=2048, D=4096, R=8, K=4):
    centroid[b] = 0.7*x[b,-1] + 0.3*mean_s x[b,s]
    w[b]        = softmax(cos(centroid[b], keys) / 0.05)
    Lam[b]      = sum_k w[b,k] * pool[k]                 # [R,R]
    out[b,s]    = gate * (x[b,s] @ V^T) @ Lam[b]^T @ U^T

Sharding: core i handles batch i//2, sequence half i%2 (1024 rows).

v2 design (vs the fp32 transpose-on-device baseline):
  - x is uploaded TRANSPOSED and in bf16: xt[d, s] per core. Every
    [128, 1024] chunk then has d on partitions, so the rank-reduce
    matmul consumes it directly -- no PE transposes at all -- and the
    full-sequence column sums for the centroid are plain free-axis
    reduces (split across DVE and the scalar engine's accumulator).
  - All matmul operands are bf16 (1 PE cycle/row, half the HBM bytes).
    Output is written bf16 and upconverted on the host.
  - Centroid is kept scale-free: cc = seqsum + (0.7*S/0.3)*x_last so
    no scaling op is needed (cosine is scale-invariant).
  - A tiny warmup AllReduce rings the ncfw doorbell during the read
    phase, cutting the real AllReduce's trigger start-delay from
    ~11.5us (cold) to ~1.5us.
  - Pairwise AllReduce of the [128, 32] partial centroid as before.

Parameter-only folding on the host: gate into U, the K mixing matrices
(gate*U @ pool[k])^T stacked as mall [32, 4096], key normalization,
V^T replicated 4x along r as vt [128, 32, 32].
"""

import numpy as np
from contextlib import ExitStack

B, S, D, R, K = 4, 2048, 4096, 8, 4
NCORES = 8
SH = S // 2            # rows per core
PT = 128               # partition tile
NCH = D // PT          # 32 d-chunks
KR = K * R             # 32
SCALE_AUX = 0.7 * S / 0.3

_CACHE = {}
LAST_RESULTS = None


def _build_program():
    from concourse import bacc, tile, mybir

    dt = mybir.dt.float32
    bf = mybir.dt.bfloat16
    add = mybir.AluOpType.add
    mult = mybir.AluOpType.mult
    AF = mybir.ActivationFunctionType
    AX = mybir.AxisListType

    nc = bacc.Bacc("TRN2", target_bir_lowering=False, debug=False,
                   num_devices=NCORES)

    GRP = 4          # chunks per read DMA; each group is a contiguous 1MB
    xt = nc.dram_tensor("xt", [NCH // GRP, PT, GRP * SH], bf,
                        kind="ExternalInput").ap()
    vt = nc.dram_tensor("vt", [PT, NCH, KR], bf, kind="ExternalInput").ap()
    mall = nc.dram_tensor("mall", [KR, D], bf, kind="ExternalInput").ap()
    kcols = nc.dram_tensor("kcols", [PT, K, NCH], dt, kind="ExternalInput").ap()
    aux = nc.dram_tensor("aux", [PT, NCH], dt, kind="ExternalInput").ap()
    mask = nc.dram_tensor("mask", [KR, K], dt, kind="ExternalInput").ap()
    out = nc.dram_tensor("out", [SH, D], bf, kind="ExternalOutput").ap()

    with tile.TileContext(nc) as tc, ExitStack() as c0:
        persist = c0.enter_context(tc.tile_pool(name="persist", bufs=1))
        dram = c0.enter_context(tc.tile_pool(name="dram", bufs=1, space="DRAM"))

        # warmup collective: rings the ncfw doorbell immediately so the
        # entry barrier + ncfw wakeup overlap the read phase; the real
        # AllReduce then starts promptly once triggered
        warm_sb = persist.tile([PT, 1], dt, name="warm_sb")
        nc.gpsimd.memset(warm_sb[:], 0.0)
        win = dram.tile([PT, 1], dt, name="win")
        wout = dram.tile([PT, 1], dt, name="wout")
        nc.gpsimd.dma_start(win[:], warm_sb[:])
        nc.gpsimd.collective_compute(
            "AllReduce",
            add,
            replica_groups=[[0, 1], [2, 3], [4, 5], [6, 7]],
            ins=[win.opt()],
            outs=[wout.opt()],
        )

        # ---- constants (gpsimd/SWDGE queue: don't block the x FIFO) ----
        vt_sb = persist.tile([PT, NCH, KR], bf, name="vt_sb")
        nc.gpsimd.dma_start(vt_sb[:], vt[:])
        mall_sb = persist.tile([KR, D], bf, name="mall_sb")
        nc.gpsimd.dma_start(mall_sb[:], mall[:])
        kcols_sb = persist.tile([PT, K, NCH], dt, name="kcols_sb")
        nc.gpsimd.dma_start(kcols_sb[:], kcols[:])
        aux_sb = persist.tile([PT, NCH], dt, name="aux_sb")
        nc.gpsimd.dma_start(aux_sb[:], aux[:])
        mask_sb = persist.tile([KR, K], dt, name="mask_sb")
        nc.gpsimd.dma_start(mask_sb[:], mask[:])
        ones_sb = persist.tile([PT, KR], dt, name="ones_sb")
        nc.vector.memset(ones_sb[:], 1.0)

        # ---- persistent intermediates ----
        seqparts = persist.tile([PT, NCH], dt, name="seqparts")
        junk_bf = persist.tile([PT, SH], bf, name="junk_bf")
        cc_sb = persist.tile([PT, NCH], dt, name="cc_sb")
        c_sb = persist.tile([PT, NCH], dt, name="c_sb")
        partials = persist.tile([PT, K + 1], dt, name="partials")
        junk = persist.tile([PT, NCH], dt, name="junk")
        xvw = persist.tile([KR, SH], bf, name="xvw")

        # ================= read phase =================
        # All 32 bf16 chunks stay resident in SBUF (64KB/partition).
        # xv[k*8+r, s] = sum_d V[r, d] x[s, d], accumulated over 32 chunks
        # into two PSUM banks (s halves), one sequential accumulation
        # group per bank. seqparts[p, ch] = sum_s x[s, dp].
        xt_all = persist.tile([PT, NCH, SH], bf, name="xt_all")
        with ExitStack() as c1:
            xvp = c1.enter_context(
                tc.tile_pool(name="xvp", bufs=1, space="PSUM"))
            xv_ps0 = xvp.tile([KR, SH // 2], dt, name="xv_ps0")
            xv_ps1 = xvp.tile([KR, SH // 2], dt, name="xv_ps1")

            for g in range(NCH // GRP):
                nc.sync.dma_start(
                    xt_all[:, g * GRP:(g + 1) * GRP, :], xt[g])
                for j in range(GRP):
                    ch = g * GRP + j
                    # seqsum over the free (s) axis: DVE reduce for even
                    # chunks, ACT copy-with-accumulator for odd chunks
                    if ch % 2 == 0:
                        nc.vector.tensor_reduce(
                            seqparts[:, ch:ch + 1], xt_all[:, ch, :],
                            axis=AX.X, op=add)
                    else:
                        nc.scalar.activation(
                            junk_bf[:], xt_all[:, ch, :], AF.Copy,
                            accum_out=seqparts[:, ch:ch + 1])
            for ch in range(NCH):
                nc.tensor.matmul(
                    xv_ps0[:], vt_sb[:, ch, :], xt_all[:, ch, 0:SH // 2],
                    start=(ch == 0), stop=(ch == NCH - 1))
            for ch in range(NCH):
                nc.tensor.matmul(
                    xv_ps1[:], vt_sb[:, ch, :], xt_all[:, ch, SH // 2:SH],
                    start=(ch == 0), stop=(ch == NCH - 1))

            # ================= routing =================
            nc.vector.tensor_add(cc_sb[:], seqparts[:], aux_sb[:])

            cin = dram.tile([PT, NCH], dt, name="cin")
            cout = dram.tile([PT, NCH], dt, name="cout")
            nc.sync.dma_start(cin[:], cc_sb[:])
            nc.gpsimd.collective_compute(
                "AllReduce",
                add,
                replica_groups=[[0, 1], [2, 3], [4, 5], [6, 7]],
                ins=[cin.opt()],
                outs=[cout.opt()],
            )
            nc.sync.dma_start(c_sb[:], cout[:])

            # per-partition partial dots <c, kn_k> (k=0..3) and |c|^2
            for k in range(K):
                nc.vector.tensor_mul(junk[:], c_sb[:], kcols_sb[:, k, :])
                nc.vector.tensor_reduce(
                    partials[:, k:k + 1], junk[:], axis=AX.X, op=add)
            nc.vector.tensor_mul(junk[:], c_sb[:], c_sb[:])
            nc.vector.tensor_reduce(
                partials[:, K:K + 1], junk[:], axis=AX.X, op=add)

            # sum over partitions + broadcast to KR rows via ones-matmul
            with tc.tile_pool(name="rps", bufs=1, space="PSUM") as rps:
                r_ps = rps.tile([KR, K + 1], dt, name="r_ps")
                nc.tensor.matmul(r_ps[:], ones_sb[:, 0:KR], partials[:],
                                 start=True, stop=True)
                rt_sb = persist.tile([KR, K + 1], dt, name="rt_sb")
                nc.scalar.copy(rt_sb[:], r_ps[:])

            # w = softmax(20 * <c,kn> / |c|)
            cn = persist.tile([KR, 1], dt, name="cn")
            nc.scalar.sqrt(cn[:], rt_sb[:, K:K + 1])
            rcn = persist.tile([KR, 1], dt, name="rcn")
            nc.vector.reciprocal(rcn[:], cn[:])
            ex = persist.tile([KR, K], dt, name="ex")
            nc.vector.tensor_scalar(ex[:], rt_sb[:, 0:K], rcn[:], 1.0 / 0.05,
                                    op0=mult, op1=mult)
            nc.scalar.activation(ex[:], ex[:], AF.Exp)
            ssum = persist.tile([KR, 1], dt, name="ssum")
            nc.vector.tensor_reduce(ssum[:], ex[:], axis=AX.X, op=add)
            rsum = persist.tile([KR, 1], dt, name="rsum")
            nc.vector.reciprocal(rsum[:], ssum[:])
            wmat = persist.tile([KR, K], dt, name="wmat")
            nc.vector.tensor_scalar_mul(wmat[:], ex[:], rsum[:])
            wcol = persist.tile([KR, 1], dt, name="wcol")
            junk2 = persist.tile([KR, K], dt, name="junk2")
            nc.vector.tensor_mul(junk2[:], wmat[:], mask_sb[:])
            nc.vector.tensor_reduce(wcol[:], junk2[:], axis=AX.X, op=add)

            # xvw[kr, s] = w_k * xv[kr, s], bf16 (ACT reads PSUM directly)
            nc.scalar.mul(xvw[:, 0:SH // 2], xv_ps0[:], wcol[:])
            nc.scalar.mul(xvw[:, SH // 2:SH], xv_ps1[:], wcol[:])

        # ================= write phase =================
        # out[t*128+s, d] = sum_kr xvw[kr, t*128+s] * mall[kr, d]
        with ExitStack() as c2:
            otp = c2.enter_context(
                tc.tile_pool(name="otp", bufs=3, space="PSUM"))
            osb_pool = c2.enter_context(tc.tile_pool(name="osb", bufs=3))

            di = 0
            for t in range(SH // PT):
                osb = osb_pool.tile([PT, D], bf, name="osb")
                for q in range(4):          # 1024-col quarter
                    o_ps = otp.tile([PT, 1024], dt, name="o_ps")
                    for h in range(2):
                        n = 2 * q + h
                        nc.tensor.matmul(
                            o_ps[:, h * 512:(h + 1) * 512],
                            xvw[:, t * PT:(t + 1) * PT],
                            mall_sb[:, n * 512:(n + 1) * 512],
                            start=True, stop=True)
                    dst = osb[:, q * 1024:(q + 1) * 1024]
                    if di % 2 == 0:
                        nc.scalar.copy(dst, o_ps[:])
                    else:
                        nc.vector.tensor_copy(dst, o_ps[:])
                    di += 1
                    if q == 1:
                        nc.sync.dma_start(
                            out[t * PT:(t + 1) * PT, 0:2048],
                            osb[:, 0:2048])
                nc.sync.dma_start(
                    out[t * PT:(t + 1) * PT, 2048:D], osb[:, 2048:D])

    nc.compile()
    return nc


def _get_program():
    if "nc" not in _CACHE:
        _CACHE["nc"] = _build_program()
    return _CACHE["nc"]


def _host_prep(x, U, V, pool, keys, gate_w, gate_b):
    """Parameter-only folding + per-core shard/aux construction."""
    import ml_dtypes
    f32 = np.float32
    bf16 = ml_dtypes.bfloat16

    # gate (parameter-only)
    gin = np.concatenate([U.mean(axis=0), V.mean(axis=1)]).astype(f32)
    z = gin @ gate_w[0].astype(f32) + gate_b[0].astype(f32)
    gate = f32(1.0) / (f32(1.0) + np.exp(-z, dtype=f32))
    Ug = (gate * U).astype(f32)

    # mall [32, 4096]: rows 8k+j = (gate*U @ pool[k])[:, j]
    mall = np.concatenate(
        [(Ug @ pool[k]).T.astype(f32) for k in range(K)], axis=0)
    mall = np.ascontiguousarray(mall).astype(bf16)

    # vt[p, c, k*R+r] = V[r, c*128+p], replicated 4x along r
    vt = np.ascontiguousarray(
        np.tile(V.T.reshape(NCH, PT, R), (1, 1, K))
        .transpose(1, 0, 2)).astype(bf16)

    # normalized keys: kcols[p, k, c] = kn[k, c*128+p]
    knorm = np.maximum(np.linalg.norm(keys, axis=1, keepdims=True), 1e-8)
    kn = (keys / knorm).astype(f32)
    kcols = np.ascontiguousarray(
        kn.reshape(K, NCH, PT).transpose(2, 0, 1))

    msk = np.zeros((KR, K), dtype=f32)
    for p in range(KR):
        msk[p, p // R] = 1.0

    shared = {"vt": vt, "mall": mall, "kcols": kcols, "mask": msk}

    x_bf = x.astype(bf16)
    in_maps = []
    for core in range(NCORES):
        b, h = divmod(core, 2)
        xh = x_bf[b, h * SH:(h + 1) * SH, :]
        xsrd = np.ascontiguousarray(
            xh.T.reshape(8, 4, PT, SH).transpose(0, 2, 1, 3)
        ).reshape(8, PT, 4 * SH)
        if h == 1:
            auxv = np.ascontiguousarray(
                (f32(SCALE_AUX) * x[b, S - 1, :]).reshape(NCH, PT).T,
                dtype=f32)
        else:
            auxv = np.zeros((PT, NCH), dtype=f32)
        in_maps.append({"xt": xsrd, "aux": auxv, **shared})
    return in_maps


def kernel(x, U_shared, V_shared, core_pool, core_keys, gate_w, gate_b):
    global LAST_RESULTS
    from concourse import bass_utils

    x = np.asarray(x, dtype=np.float32)
    U = np.asarray(U_shared, dtype=np.float32)
    V = np.asarray(V_shared, dtype=np.float32)
    pool = np.asarray(core_pool, dtype=np.float32)
    keys = np.asarray(core_keys, dtype=np.float32)
    gw = np.asarray(gate_w, dtype=np.float32)
    gb = np.asarray(gate_b, dtype=np.float32)

    nc = _get_program()
    in_maps = _host_prep(x, U, V, pool, keys, gw, gb)
    res = bass_utils.run_bass_kernel_spmd(
        nc, in_maps, core_ids=list(range(NCORES)))
    LAST_RESULTS = res

    out = np.empty((B, S, D), dtype=np.float32)
    for core in range(NCORES):
        b, h = divmod(core, 2)
        out[b, h * SH:(h + 1) * SH, :] = res.results[core]["out"]
    return out


# revision 18
# speedup vs baseline: 1.3443x; 1.3443x over previous
"""CASCADES adapter (moe_routing) on 8 TRN2 NeuronCores.

Reference computation (B=4, S=2048, D=4096, R=8, K=4):
    centroid[b] = 0.7*x[b,-1] + 0.3*mean_s x[b,s]
    w[b]        = softmax(cos(centroid[b], keys) / 0.05)
    Lam[b]      = sum_k w[b,k] * pool[k]                 # [R,R]
    out[b,s]    = gate * (x[b,s] @ V^T) @ Lam[b]^T @ U^T

Sharding: core i handles batch i//2, sequence half i%2 (1024 rows).

v2 design (vs the fp32 transpose-on-device baseline):
  - x is uploaded TRANSPOSED and in bf16: xt[d, s] per core. Every
    [128, 1024] chunk then has d on partitions, so the rank-reduce
    matmul consumes it directly -- no PE transposes at all -- and the
    full-sequence column sums for the centroid are plain free-axis
    reduces (split across DVE and the scalar engine's accumulator).
  - All matmul operands are bf16 (1 PE cycle/row, half the HBM bytes).
    Output is written bf16 and upconverted on the host.
  - Centroid is kept scale-free: cc = seqsum + (0.7*S/0.3)*x_last so
    no scaling op is needed (cosine is scale-invariant).
  - A tiny warmup AllReduce rings the ncfw doorbell during the read
    phase, cutting the real AllReduce's trigger start-delay from
    ~11.5us (cold) to ~1.5us.
  - Pairwise AllReduce of the [128, 32] partial centroid as before.

Parameter-only folding on the host: gate into U, the K mixing matrices
(gate*U @ pool[k])^T stacked as mall [32, 4096], key normalization,
V^T replicated 4x along r as vt [128, 32, 32].
"""

import numpy as np
from contextlib import ExitStack

B, S, D, R, K = 4, 2048, 4096, 8, 4
NCORES = 8
SH = S // 2            # rows per core
PT = 128               # partition tile
NCH = D // PT          # 32 d-chunks
KR = K * R             # 32
SCALE_AUX = 0.7 * S / 0.3

_CACHE = {}
LAST_RESULTS = None


def _build_program():
    from concourse import bacc, tile, mybir

    dt = mybir.dt.float32
    bf = mybir.dt.bfloat16
    add = mybir.AluOpType.add
    mult = mybir.AluOpType.mult
    AF = mybir.ActivationFunctionType
    AX = mybir.AxisListType

    nc = bacc.Bacc("TRN2", target_bir_lowering=False, debug=False,
                   num_devices=NCORES)

    GRP = 4          # chunks per read DMA; each group is a contiguous 1MB
    xt = nc.dram_tensor("xt", [NCH // GRP, PT, GRP * SH], bf,
                        kind="ExternalInput").ap()
    vt = nc.dram_tensor("vt", [PT, NCH, KR], bf, kind="ExternalInput").ap()
    mall = nc.dram_tensor("mall", [KR, D], bf, kind="ExternalInput").ap()
    kcols = nc.dram_tensor("kcols", [PT, K, NCH], dt, kind="ExternalInput").ap()
    aux = nc.dram_tensor("aux", [PT, NCH], dt, kind="ExternalInput").ap()
    mask = nc.dram_tensor("mask", [KR, K], dt, kind="ExternalInput").ap()
    out = nc.dram_tensor("out", [SH, D], bf, kind="ExternalOutput").ap()

    with tile.TileContext(nc) as tc, ExitStack() as c0:
        persist = c0.enter_context(tc.tile_pool(name="persist", bufs=1))
        dram = c0.enter_context(tc.tile_pool(name="dram", bufs=1, space="DRAM"))

        # warmup collective: rings the ncfw doorbell immediately so the
        # entry barrier + ncfw wakeup overlap the read phase; the real
        # AllReduce then starts promptly once triggered
        # the input is an unwritten scratch tile: the warmup's result is
        # never read, only its completion timing matters, and the trigger
        # must be the FIRST gpsimd instruction (a collective trigger blocks
        # the gpsimd queue until the collective completes)
        win = dram.tile([PT, 1], dt, name="win")
        wout = dram.tile([PT, 1], dt, name="wout")
        nc.gpsimd.collective_compute(
            "AllReduce",
            add,
            replica_groups=[[0, 1], [2, 3], [4, 5], [6, 7]],
            ins=[win.opt()],
            outs=[wout.opt()],
        )

        # ---- constants (gpsimd/SWDGE queue: don't block the x FIFO) ----
        vt_sb = persist.tile([PT, NCH, KR], bf, name="vt_sb")
        nc.gpsimd.dma_start(vt_sb[:], vt[:])
        mall_sb = persist.tile([KR, D], bf, name="mall_sb")
        nc.gpsimd.dma_start(mall_sb[:], mall[:])
        kcols_sb = persist.tile([PT, K, NCH], dt, name="kcols_sb")
        nc.gpsimd.dma_start(kcols_sb[:], kcols[:])
        aux_sb = persist.tile([PT, NCH], dt, name="aux_sb")
        nc.gpsimd.dma_start(aux_sb[:], aux[:])
        mask_sb = persist.tile([KR, K], dt, name="mask_sb")
        nc.gpsimd.dma_start(mask_sb[:], mask[:])
        ones_sb = persist.tile([PT, KR], dt, name="ones_sb")
        nc.vector.memset(ones_sb[:], 1.0)

        # ---- persistent intermediates ----
        seqparts = persist.tile([PT, NCH], dt, name="seqparts")
        junk_bf = persist.tile([PT, SH], bf, name="junk_bf")
        cc_sb = persist.tile([PT, NCH], dt, name="cc_sb")
        c_sb = persist.tile([PT, NCH], dt, name="c_sb")
        partials = persist.tile([PT, K + 1], dt, name="partials")
        junk = persist.tile([PT, NCH], dt, name="junk")
        xvw = persist.tile([KR, SH], bf, name="xvw")

        # ================= read phase =================
        # All 32 bf16 chunks stay resident in SBUF (64KB/partition).
        # xv[k*8+r, s] = sum_d V[r, d] x[s, d], accumulated over 32 chunks
        # into two PSUM banks (s halves), one sequential accumulation
        # group per bank. seqparts[p, ch] = sum_s x[s, dp].
        xt_all = persist.tile([PT, NCH, SH], bf, name="xt_all")
        with ExitStack() as c1:
            xvp = c1.enter_context(
                tc.tile_pool(name="xvp", bufs=1, space="PSUM"))
            xv_ps0 = xvp.tile([KR, SH // 2], dt, name="xv_ps0")
            xv_ps1 = xvp.tile([KR, SH // 2], dt, name="xv_ps1")

            for g in range(NCH // GRP):
                nc.sync.dma_start(
                    xt_all[:, g * GRP:(g + 1) * GRP, :], xt[g])
                for j in range(GRP):
                    ch = g * GRP + j
                    # seqsum over the free (s) axis: DVE reduce for even
                    # chunks, ACT copy-with-accumulator for odd chunks
                    if ch % 2 == 0:
                        nc.vector.tensor_reduce(
                            seqparts[:, ch:ch + 1], xt_all[:, ch, :],
                            axis=AX.X, op=add)
                    else:
                        nc.scalar.activation(
                            junk_bf[:], xt_all[:, ch, :], AF.Copy,
                            accum_out=seqparts[:, ch:ch + 1])
            for ch in range(NCH):
                nc.tensor.matmul(
                    xv_ps0[:], vt_sb[:, ch, :], xt_all[:, ch, 0:SH // 2],
                    start=(ch == 0), stop=(ch == NCH - 1))
            for ch in range(NCH):
                nc.tensor.matmul(
                    xv_ps1[:], vt_sb[:, ch, :], xt_all[:, ch, SH // 2:SH],
                    start=(ch == 0), stop=(ch == NCH - 1))

            # ================= routing =================
            nc.vector.tensor_add(cc_sb[:], seqparts[:], aux_sb[:])

            cin = dram.tile([PT, NCH], dt, name="cin")
            cout = dram.tile([PT, NCH], dt, name="cout")
            nc.sync.dma_start(cin[:], cc_sb[:])
            nc.gpsimd.collective_compute(
                "AllReduce",
                add,
                replica_groups=[[0, 1], [2, 3], [4, 5], [6, 7]],
                ins=[cin.opt()],
                outs=[cout.opt()],
            )
            nc.sync.dma_start(c_sb[:], cout[:])

            # per-partition partial dots <c, kn_k> (k=0..3) and |c|^2
            for k in range(K):
                nc.vector.tensor_mul(junk[:], c_sb[:], kcols_sb[:, k, :])
                nc.vector.tensor_reduce(
                    partials[:, k:k + 1], junk[:], axis=AX.X, op=add)
            nc.vector.tensor_mul(junk[:], c_sb[:], c_sb[:])
            nc.vector.tensor_reduce(
                partials[:, K:K + 1], junk[:], axis=AX.X, op=add)

            # sum over partitions + broadcast to KR rows via ones-matmul
            with tc.tile_pool(name="rps", bufs=1, space="PSUM") as rps:
                r_ps = rps.tile([KR, K + 1], dt, name="r_ps")
                nc.tensor.matmul(r_ps[:], ones_sb[:, 0:KR], partials[:],
                                 start=True, stop=True)
                rt_sb = persist.tile([KR, K + 1], dt, name="rt_sb")
                nc.scalar.copy(rt_sb[:], r_ps[:])

            # w = softmax(20 * <c,kn> / |c|)
            cn = persist.tile([KR, 1], dt, name="cn")
            nc.scalar.sqrt(cn[:], rt_sb[:, K:K + 1])
            rcn = persist.tile([KR, 1], dt, name="rcn")
            nc.vector.reciprocal(rcn[:], cn[:])
            ex = persist.tile([KR, K], dt, name="ex")
            nc.vector.tensor_scalar(ex[:], rt_sb[:, 0:K], rcn[:], 1.0 / 0.05,
                                    op0=mult, op1=mult)
            nc.scalar.activation(ex[:], ex[:], AF.Exp)
            ssum = persist.tile([KR, 1], dt, name="ssum")
            nc.vector.tensor_reduce(ssum[:], ex[:], axis=AX.X, op=add)
            rsum = persist.tile([KR, 1], dt, name="rsum")
            nc.vector.reciprocal(rsum[:], ssum[:])
            wmat = persist.tile([KR, K], dt, name="wmat")
            nc.vector.tensor_scalar_mul(wmat[:], ex[:], rsum[:])
            wcol = persist.tile([KR, 1], dt, name="wcol")
            junk2 = persist.tile([KR, K], dt, name="junk2")
            nc.vector.tensor_mul(junk2[:], wmat[:], mask_sb[:])
            nc.vector.tensor_reduce(wcol[:], junk2[:], axis=AX.X, op=add)

            # xvw[kr, s] = w_k * xv[kr, s], bf16 (ACT reads PSUM directly)
            nc.scalar.mul(xvw[:, 0:SH // 2], xv_ps0[:], wcol[:])
            nc.scalar.mul(xvw[:, SH // 2:SH], xv_ps1[:], wcol[:])

        # ================= write phase =================
        # out[t*128+s, d] = sum_kr xvw[kr, t*128+s] * mall[kr, d]
        with ExitStack() as c2:
            otp = c2.enter_context(
                tc.tile_pool(name="otp", bufs=3, space="PSUM"))
            osb_pool = c2.enter_context(tc.tile_pool(name="osb", bufs=3))

            di = 0
            for t in range(SH // PT):
                osb = osb_pool.tile([PT, D], bf, name="osb")
                for q in range(4):          # 1024-col quarter
                    o_ps = otp.tile([PT, 1024], dt, name="o_ps")
                    for h in range(2):
                        n = 2 * q + h
                        nc.tensor.matmul(
                            o_ps[:, h * 512:(h + 1) * 512],
                            xvw[:, t * PT:(t + 1) * PT],
                            mall_sb[:, n * 512:(n + 1) * 512],
                            start=True, stop=True)
                    dst = osb[:, q * 1024:(q + 1) * 1024]
                    if di % 2 == 0:
                        nc.scalar.copy(dst, o_ps[:])
                    else:
                        nc.vector.tensor_copy(dst, o_ps[:])
                    di += 1
                    if q == 1:
                        nc.sync.dma_start(
                            out[t * PT:(t + 1) * PT, 0:2048],
                            osb[:, 0:2048])
                nc.sync.dma_start(
                    out[t * PT:(t + 1) * PT, 2048:D], osb[:, 2048:D])

    nc.compile()
    return nc


def _get_program():
    if "nc" not in _CACHE:
        _CACHE["nc"] = _build_program()
    return _CACHE["nc"]


def _host_prep(x, U, V, pool, keys, gate_w, gate_b):
    """Parameter-only folding + per-core shard/aux construction."""
    import ml_dtypes
    f32 = np.float32
    bf16 = ml_dtypes.bfloat16

    # gate (parameter-only)
    gin = np.concatenate([U.mean(axis=0), V.mean(axis=1)]).astype(f32)
    z = gin @ gate_w[0].astype(f32) + gate_b[0].astype(f32)
    gate = f32(1.0) / (f32(1.0) + np.exp(-z, dtype=f32))
    Ug = (gate * U).astype(f32)

    # mall [32, 4096]: rows 8k+j = (gate*U @ pool[k])[:, j]
    mall = np.concatenate(
        [(Ug @ pool[k]).T.astype(f32) for k in range(K)], axis=0)
    mall = np.ascontiguousarray(mall).astype(bf16)

    # vt[p, c, k*R+r] = V[r, c*128+p], replicated 4x along r
    vt = np.ascontiguousarray(
        np.tile(V.T.reshape(NCH, PT, R), (1, 1, K))
        .transpose(1, 0, 2)).astype(bf16)

    # normalized keys: kcols[p, k, c] = kn[k, c*128+p]
    knorm = np.maximum(np.linalg.norm(keys, axis=1, keepdims=True), 1e-8)
    kn = (keys / knorm).astype(f32)
    kcols = np.ascontiguousarray(
        kn.reshape(K, NCH, PT).transpose(2, 0, 1))

    msk = np.zeros((KR, K), dtype=f32)
    for p in range(KR):
        msk[p, p // R] = 1.0

    shared = {"vt": vt, "mall": mall, "kcols": kcols, "mask": msk}

    x_bf = x.astype(bf16)
    in_maps = []
    for core in range(NCORES):
        b, h = divmod(core, 2)
        xh = x_bf[b, h * SH:(h + 1) * SH, :]
        xsrd = np.ascontiguousarray(
            xh.T.reshape(8, 4, PT, SH).transpose(0, 2, 1, 3)
        ).reshape(8, PT, 4 * SH)
        if h == 1:
            auxv = np.ascontiguousarray(
                (f32(SCALE_AUX) * x[b, S - 1, :]).reshape(NCH, PT).T,
                dtype=f32)
        else:
            auxv = np.zeros((PT, NCH), dtype=f32)
        in_maps.append({"xt": xsrd, "aux": auxv, **shared})
    return in_maps


def kernel(x, U_shared, V_shared, core_pool, core_keys, gate_w, gate_b):
    global LAST_RESULTS
    from concourse import bass_utils

    x = np.asarray(x, dtype=np.float32)
    U = np.asarray(U_shared, dtype=np.float32)
    V = np.asarray(V_shared, dtype=np.float32)
    pool = np.asarray(core_pool, dtype=np.float32)
    keys = np.asarray(core_keys, dtype=np.float32)
    gw = np.asarray(gate_w, dtype=np.float32)
    gb = np.asarray(gate_b, dtype=np.float32)

    nc = _get_program()
    in_maps = _host_prep(x, U, V, pool, keys, gw, gb)
    res = bass_utils.run_bass_kernel_spmd(
        nc, in_maps, core_ids=list(range(NCORES)))
    LAST_RESULTS = res

    out = np.empty((B, S, D), dtype=np.float32)
    for core in range(NCORES):
        b, h = divmod(core, 2)
        out[b, h * SH:(h + 1) * SH, :] = res.results[core]["out"]
    return out
